# revision 37
# baseline (speedup 1.0000x reference)
"""Bahdanau attention Trainium2 kernel.

reference:
    q = query @ Wq.T                    # [B,1,H]
    e = tanh(q + proj_key)              # [B,S,H]
    scores = e @ we                     # [B,S]
    scores = where(mask==0, -inf, scores)
    alphas = softmax(scores, -1)        # [B,S]
    context = alphas @ value            # [B,1,2H]
    returns (context[:,None,:], alphas[:,None,:])

Sharding: B=16 batches data-parallel over 8 cores (2 per core). Params
replicated. Sparse mode: only rows with mask==1 are gathered from DRAM
(masked rows have alpha identically 0 and contribute nothing), roughly
halving HBM traffic, which is the bottleneck.
"""

import sys
import types
from contextlib import ExitStack

import numpy as np

# ---------------------------------------------------------------- axon shim
_hook = [None]


def _install_axon_ntff_hook():
    if "antenv.axon_hooks" not in sys.modules:
        mod = types.ModuleType("antenv.axon_hooks")
        mod.set_axon_ntff_profile_hook = lambda h: _hook.__setitem__(0, h)
        mod.get_axon_ntff_profile_hook = lambda: _hook[0]
        sys.modules["antenv.axon_hooks"] = mod
        try:
            import antenv

            antenv.axon_hooks = mod
        except ImportError:
            pass
    if _hook[0] is None:
        try:
            from trn_agent_boot.trn_boot import _ntff_profile_via_ctypes

            _hook[0] = _ntff_profile_via_ctypes("/opt/axon/libaxon_pjrt.so")
        except Exception:
            pass


_install_axon_ntff_hook()

import concourse.bass as bass
import concourse.tile as tile
from concourse import bacc, mybir
from concourse.bass import ds
from concourse.bass_utils import run_bass_kernel_spmd
from concourse.masks import make_identity

F32 = mybir.dt.float32
BF16 = mybir.dt.bfloat16
I32 = mybir.dt.int32
AF = mybir.ActivationFunctionType
ALU = mybir.AluOpType

B, S, H = 16, 4096, 1024
D = 2 * H
N_CORES = 8
BL = B // N_CORES  # batches per core
HC = H // 128  # h-chunks
NEG = -1.0e30


def build_program(T: int, sparse: bool):
    """One SPMD Bass program for BL batches with T s-tiles of 128 each."""
    nc = bacc.Bacc(
        "TRN2",
        target_bir_lowering=False,
        debug=False,
        enable_asserts=False,
        num_devices=N_CORES,
    )
    pk = nc.dram_tensor("pk", [BL * S, H], F32, kind="ExternalInput").ap()
    val = nc.dram_tensor("val", [BL * S, D], F32, kind="ExternalInput").ap()
    wqt = nc.dram_tensor("wqt", [H, H], F32, kind="ExternalInput").ap()
    qcol = nc.dram_tensor("qcol", [BL, 128, HC], F32, kind="ExternalInput").ap()
    werep = nc.dram_tensor("werep", [128, H], F32, kind="ExternalInput").ap()
    nmask = nc.dram_tensor("nmask", [BL, 128, T], F32, kind="ExternalInput").ap()
    if sparse:
        idx = nc.dram_tensor("idx", [BL, 128, T], I32, kind="ExternalInput").ap()
    ctx_out = nc.dram_tensor("ctx", [BL, D], F32, kind="ExternalOutput").ap()
    alphat = nc.dram_tensor("alphat", [BL, T, 128], F32, kind="ExternalOutput").ap()

    with tile.TileContext(nc) as tc, ExitStack() as ctx:
        const = ctx.enter_context(tc.tile_pool(name="const", bufs=1))
        pk_pool = ctx.enter_context(tc.tile_pool(name="pk", bufs=10))
        v_pool = ctx.enter_context(tc.tile_pool(name="v", bufs=8))
        sum_pool = ctx.enter_context(tc.tile_pool(name="sum", bufs=4))
        e_pool = ctx.enter_context(tc.tile_pool(name="e", bufs=4))
        pr_pool = ctx.enter_context(tc.tile_pool(name="pr", bufs=2))
        sm_pool = ctx.enter_context(tc.tile_pool(name="sm", bufs=2))
        out_pool = ctx.enter_context(tc.tile_pool(name="out", bufs=2))
        qps_pool = ctx.enter_context(tc.tile_pool(name="qps", bufs=1, space="PSUM"))
        cps_pool = ctx.enter_context(tc.tile_pool(name="cps", bufs=1, space="PSUM"))
        sps_pool = ctx.enter_context(tc.tile_pool(name="sps", bufs=2, space="PSUM"))

        # ---- constants (idx first: the gathers gate on it; wqt is big
        # and only needed by qproj, so it loads last)
        if sparse:
            idx_sb = const.tile([128, BL * T], I32, tag="idx")
            nc.sync.dma_start(
                idx_sb[:].rearrange("p (b t) -> p b t", t=T),
                idx.rearrange("b p t -> p b t"),
            )
        qcol_sb = const.tile([128, BL * HC], F32, tag="qcol")
        nc.sync.dma_start(
            qcol_sb[:].rearrange("p (b c) -> p b c", c=HC),
            qcol.rearrange("b p c -> p b c"),
        )
        nmask_sb = const.tile([128, BL * T], F32, tag="nmask")
        nc.sync.dma_start(
            nmask_sb[:].rearrange("p (b t) -> p b t", t=T),
            nmask.rearrange("b p t -> p b t"),
        )
        werep_sb = const.tile([128, H], F32, tag="werep")
        nc.sync.dma_start(werep_sb[:], werep[:])
        wqt_sb = const.tile([128, HC * H], F32, tag="wqt")
        nc.sync.dma_start(
            wqt_sb[:].rearrange("p (c o) -> p c o", o=H),
            wqt.rearrange("(c p) o -> p c o", p=128),
        )
        ident_sb = const.tile([128, 128], F32, tag="ident")
        ones_col = const.tile([128, 1], F32, tag="ones_col")
        ones_row = const.tile([1, 128], F32, tag="ones_row")

        def make_consts():
            # emitted after the first gather burst so these gpsimd ops
            # don't sit ahead of the gathers in the queue
            make_identity(nc, ident_sb[:])
            nc.gpsimd.memset(ones_col[:], 1.0)
            nc.gpsimd.memset(ones_row[:], 1.0)

        def qproj(b):
            # q_rep[p, o] = sum_h query[h] * WqT[h, o]   (same for all p)
            qps = qps_pool.tile([128, H], F32, tag="qps")
            for c in range(HC):
                lhsT = qcol_sb[:, b * HC + c : b * HC + c + 1].to_broadcast([128, 128])
                for hh in range(2):
                    nc.tensor.matmul(
                        qps[:, hh * 512 : (hh + 1) * 512],
                        lhsT=lhsT,
                        rhs=wqt_sb[:, c * H + hh * 512 : c * H + (hh + 1) * 512],
                        start=(c == 0),
                        stop=(c == HC - 1),
                    )
            qrep = sm_pool.tile([128, H], F32, tag="qrep")
            nc.scalar.copy(qrep[:], qps[:])
            return qrep

        def fused_tile(b, t, qrep, scores, p_col, p_bf, cps):
            """One s-tile: gather pk+v, score it, exp (mask folded into
            the per-partition exp bias), bf16-cast, and accumulate the
            context matmul in PSUM — fully online softmax (skipping the
            max-subtraction is safe: |scores| <= sum|we| keeps exp in
            fp32 range, matching the reference softmax exactly)."""
            pk_t = pk_pool.tile([128, H], F32, tag="pk")
            if sparse:
                nc.gpsimd.indirect_dma_start(
                    out=pk_t[:],
                    out_offset=None,
                    in_=pk[:],
                    in_offset=bass.IndirectOffsetOnAxis(
                        ap=idx_sb[:, b * T + t : b * T + t + 1], axis=0
                    ),
                )
            else:
                nc.sync.dma_start(pk_t[:], pk[ds(b * S + t * 128, 128), :])
            v_t = v_pool.tile([128, D], BF16, tag="v")
            if sparse:
                nc.gpsimd.indirect_dma_start(
                    out=v_t[:],
                    out_offset=None,
                    in_=val[:],
                    in_offset=bass.IndirectOffsetOnAxis(
                        ap=idx_sb[:, b * T + t : b * T + t + 1], axis=0
                    ),
                )
            else:
                nc.gpsimd.dma_start(v_t[:], val[ds(b * S + t * 128, 128), :])
            s_t = sum_pool.tile([128, H], F32, tag="sum")
            nc.vector.tensor_add(s_t[:], pk_t[:], qrep[:])
            e_t = e_pool.tile([128, H], F32, tag="e")
            nc.scalar.activation(e_t[:], s_t[:], AF.Tanh)
            pr_t = pr_pool.tile([128, H], F32, tag="pr")
            nc.vector.scalar_tensor_tensor(
                out=pr_t[:],
                in0=e_t[:],
                scalar=0.0,
                in1=werep_sb[:],
                op0=ALU.bypass,
                op1=ALU.mult,
                accum_out=scores[:, t : t + 1],
            )
            nc.scalar.activation(
                p_col[:, t : t + 1],
                scores[:, t : t + 1],
                AF.Exp,
                bias=nmask_sb[:, b * T + t : b * T + t + 1],
            )
            nc.vector.tensor_copy(p_bf[:, t : t + 1], p_col[:, t : t + 1])
            for cc in range(D // 512):
                nc.tensor.matmul(
                    cps[:, cc * 512 : (cc + 1) * 512],
                    lhsT=p_bf[:, t : t + 1],
                    rhs=v_t[:, cc * 512 : (cc + 1) * 512],
                    start=(t == 0),
                    stop=(t == T - 1),
                )

        def finish_batch(b, p_col, cps):
            rowsum = sm_pool.tile([128, 1], F32, tag="rowsum")
            nc.vector.reduce_sum(rowsum[:], p_col[:], axis=mybir.AxisListType.X)
            zps = sps_pool.tile([1, 1], F32, tag="sps")
            nc.tensor.matmul(
                zps[:], lhsT=rowsum[:], rhs=ones_col[:], start=True, stop=True
            )
            rz = sm_pool.tile([1, 1], F32, tag="rz")
            nc.vector.reciprocal(rz[:], zps[:])
            rzcps = sps_pool.tile([128, 1], F32, tag="sps")
            nc.tensor.matmul(
                rzcps[:], lhsT=ones_row[:], rhs=rz[:], start=True, stop=True
            )
            rzc = sm_pool.tile([128, 1], F32, tag="rzc")
            nc.vector.tensor_copy(rzc[:], rzcps[:])

            ctx_sb = out_pool.tile([1, D], F32, tag="ctx_sb")
            nc.scalar.mul(ctx_sb[:], cps[:], rz[0:1, 0:1])
            nc.sync.dma_start(ctx_out[b : b + 1, :], ctx_sb[:])
            aps = sps_pool.tile([T, 128], F32, tag="sps")
            nc.tensor.transpose(out=aps[:], in_=p_col[:], identity=ident_sb[:])
            at_sb = out_pool.tile([T, 128], F32, tag="at_sb")
            nc.scalar.mul(at_sb[:], aps[:], rzc[0:T, 0:1])
            nc.sync.dma_start(alphat[b, :, :], at_sb[:])

        qreps = [qproj(b) for b in range(BL)]
        for b in range(BL):
            scores = sm_pool.tile([128, T], F32, tag="scores")
            p_col = sm_pool.tile([128, T], F32, tag="pcol")
            p_bf = sm_pool.tile([128, T], BF16, tag="pbf")
            cps = cps_pool.tile([1, D], F32, tag="cps")
            for t in range(T):
                fused_tile(b, t, qreps[b], scores, p_col, p_bf, cps)
            if b == 0:
                make_consts()
            finish_batch(b, p_col, cps)

    nc.compile()
    return nc


_program_cache = {}


def _get_program(T: int, sparse: bool):
    key = (T, sparse)
    if key not in _program_cache:
        _program_cache[key] = build_program(T, sparse)
    return _program_cache[key]


def kernel(query, proj_key, value, Wq, we, mask, sparse=True, trace=False):
    query = np.asarray(query, dtype=np.float32)
    proj_key = np.asarray(proj_key, dtype=np.float32)
    value = np.asarray(value, dtype=np.float32)
    Wq = np.asarray(Wq, dtype=np.float32)
    we = np.asarray(we, dtype=np.float32)
    mask = np.asarray(mask)
    mask_i = np.asarray(mask, dtype=np.int64)

    # shared host-side layout prep (replicated params)
    wqt_np = np.ascontiguousarray(Wq.T)
    werep_np = np.ascontiguousarray(np.broadcast_to(we, (128, H)))

    if sparse:
        idx_lists = [np.nonzero(mask_i[g] != 0)[0].astype(np.int32) for g in range(B)]
        kmax = max(1, max(len(ix) for ix in idx_lists))
        T = -(-kmax // 128) * 128 // 128  # ceil to tiles
    else:
        T = S // 128

    nc = _get_program(T, sparse)

    in_maps = []
    for c in range(N_CORES):
        g0 = c * BL
        pk_np = np.ascontiguousarray(proj_key[g0 : g0 + BL].reshape(BL * S, H))
        v_np = np.ascontiguousarray(value[g0 : g0 + BL].reshape(BL * S, D))
        qcol_np = np.ascontiguousarray(
            query[g0 : g0 + BL, 0, :].reshape(BL, HC, 128).transpose(0, 2, 1)
        )
        nm = np.full((BL, T * 128), NEG, dtype=np.float32)
        ix = np.zeros((BL, T * 128), dtype=np.int32)
        for i in range(BL):
            g = g0 + i
            if sparse:
                k = len(idx_lists[g])
                nm[i, :k] = 0.0
                ix[i, :k] = g0 * 0 + i * S + idx_lists[g]
                ix[i, k:] = i * S  # harmless valid row; weight is exactly 0
            else:
                nm[i] = np.where(mask_i[g] != 0, 0.0, NEG).astype(np.float32)
        nm_col = np.ascontiguousarray(
            nm.reshape(BL, T, 128).transpose(0, 2, 1)
        )
        m = {
            "pk": pk_np,
            "val": v_np,
            "wqt": wqt_np,
            "qcol": qcol_np,
            "werep": werep_np,
            "nmask": nm_col,
        }
        if sparse:
            m["idx"] = np.ascontiguousarray(ix.reshape(BL, T, 128).transpose(0, 2, 1))
        in_maps.append(m)

    res = run_bass_kernel_spmd(
        nc, in_maps, core_ids=list(range(N_CORES)), trace=trace
    )

    context = np.zeros((B, 1, D), dtype=np.float32)
    alphas = np.zeros((B, 1, S), dtype=np.float32)
    for c in range(N_CORES):
        r = res.results[c]
        for i in range(BL):
            g = c * BL + i
            context[g, 0, :] = r["ctx"][i]
            vals = r["alphat"][i].reshape(T * 128)
            if sparse:
                k = len(idx_lists[g])
                alphas[g, 0, idx_lists[g]] = vals[:k]
            else:
                alphas[g, 0, :] = vals
    kernel.last_exec_time_ns = res.exec_time_ns
    kernel.last_results = res
    return context, alphas


# revision 38
# speedup vs baseline: 1.8799x; 1.8799x over previous
"""Bahdanau attention Trainium2 kernel.

reference:
    q = query @ Wq.T                    # [B,1,H]
    e = tanh(q + proj_key)              # [B,S,H]
    scores = e @ we                     # [B,S]
    scores = where(mask==0, -inf, scores)
    alphas = softmax(scores, -1)        # [B,S]
    context = alphas @ value            # [B,1,2H]
    returns (context[:,None,:], alphas[:,None,:])

Sharding: B=16 batches data-parallel over 8 cores (2 per core). Params
replicated. Sparse mode: only rows with mask==1 are gathered from DRAM
(masked rows have alpha identically 0 and contribute nothing), roughly
halving HBM traffic, which is the bottleneck.
"""

import sys
import types
from contextlib import ExitStack

import numpy as np

# ---------------------------------------------------------------- axon shim
_hook = [None]


def _install_axon_ntff_hook():
    if "antenv.axon_hooks" not in sys.modules:
        mod = types.ModuleType("antenv.axon_hooks")
        mod.set_axon_ntff_profile_hook = lambda h: _hook.__setitem__(0, h)
        mod.get_axon_ntff_profile_hook = lambda: _hook[0]
        sys.modules["antenv.axon_hooks"] = mod
        try:
            import antenv

            antenv.axon_hooks = mod
        except ImportError:
            pass
    if _hook[0] is None:
        try:
            from trn_agent_boot.trn_boot import _ntff_profile_via_ctypes

            _hook[0] = _ntff_profile_via_ctypes("/opt/axon/libaxon_pjrt.so")
        except Exception:
            pass


_install_axon_ntff_hook()

import concourse.bass as bass
import concourse.tile as tile
from concourse import bacc, mybir
from concourse.bass import ds
from concourse.bass_utils import run_bass_kernel_spmd
from concourse.masks import make_identity

F32 = mybir.dt.float32
F16 = mybir.dt.float16  # context operands: 2-byte PE speed, 11-bit mantissa
I32 = mybir.dt.int32
AF = mybir.ActivationFunctionType
ALU = mybir.AluOpType

B, S, H = 16, 4096, 1024
D = 2 * H
N_CORES = 8
BL = B // N_CORES  # batches per core
HC = H // 128  # h-chunks
NEG = -1.0e30


def build_program(T: int, sparse: bool):
    """One SPMD Bass program for BL batches with T s-tiles of 128 each."""
    nc = bacc.Bacc(
        "TRN2",
        target_bir_lowering=False,
        debug=False,
        enable_asserts=False,
        num_devices=N_CORES,
    )
    pk = nc.dram_tensor("pk", [BL * S, H], F32, kind="ExternalInput").ap()
    val = nc.dram_tensor("val", [BL * S, D], F32, kind="ExternalInput").ap()
    wqt = nc.dram_tensor("wqt", [H, H], F32, kind="ExternalInput").ap()
    qcol = nc.dram_tensor("qcol", [BL, 128, HC], F32, kind="ExternalInput").ap()
    werep = nc.dram_tensor("werep", [128, H], F32, kind="ExternalInput").ap()
    nmask = nc.dram_tensor("nmask", [BL, 128, T], F32, kind="ExternalInput").ap()
    if sparse:
        idx = nc.dram_tensor("idx", [BL, 128, T], I32, kind="ExternalInput").ap()
    ctx_out = nc.dram_tensor("ctx", [BL, D], F32, kind="ExternalOutput").ap()
    alphat = nc.dram_tensor("alphat", [BL, T, 128], F32, kind="ExternalOutput").ap()

    with tile.TileContext(nc) as tc, ExitStack() as ctx:
        const = ctx.enter_context(tc.tile_pool(name="const", bufs=1))
        pk_pool = ctx.enter_context(tc.tile_pool(name="pk", bufs=10))
        v_pool = ctx.enter_context(tc.tile_pool(name="v", bufs=8))
        sum_pool = ctx.enter_context(tc.tile_pool(name="sum", bufs=4))
        e_pool = ctx.enter_context(tc.tile_pool(name="e", bufs=4))
        pr_pool = ctx.enter_context(tc.tile_pool(name="pr", bufs=2))
        sm_pool = ctx.enter_context(tc.tile_pool(name="sm", bufs=2))
        out_pool = ctx.enter_context(tc.tile_pool(name="out", bufs=2))
        qps_pool = ctx.enter_context(tc.tile_pool(name="qps", bufs=1, space="PSUM"))
        cps_pool = ctx.enter_context(tc.tile_pool(name="cps", bufs=1, space="PSUM"))
        sps_pool = ctx.enter_context(tc.tile_pool(name="sps", bufs=2, space="PSUM"))

        # ---- constants (idx first: the gathers gate on it; wqt is big
        # and only needed by qproj, so it loads last)
        if sparse:
            idx_sb = const.tile([128, BL * T], I32, tag="idx")
            nc.sync.dma_start(
                idx_sb[:].rearrange("p (b t) -> p b t", t=T),
                idx.rearrange("b p t -> p b t"),
            )
        qcol_sb = const.tile([128, BL * HC], F32, tag="qcol")
        nc.sync.dma_start(
            qcol_sb[:].rearrange("p (b c) -> p b c", c=HC),
            qcol.rearrange("b p c -> p b c"),
        )
        nmask_sb = const.tile([128, BL * T], F32, tag="nmask")
        nc.sync.dma_start(
            nmask_sb[:].rearrange("p (b t) -> p b t", t=T),
            nmask.rearrange("b p t -> p b t"),
        )
        werep_sb = const.tile([128, H], F32, tag="werep")
        nc.sync.dma_start(werep_sb[:], werep[:])
        wqt_sb = const.tile([128, HC * H], F32, tag="wqt")
        nc.sync.dma_start(
            wqt_sb[:].rearrange("p (c o) -> p c o", o=H),
            wqt.rearrange("(c p) o -> p c o", p=128),
        )
        ident_sb = const.tile([128, 128], F32, tag="ident")
        ones_col = const.tile([128, 1], F32, tag="ones_col")
        ones_row = const.tile([1, 128], F32, tag="ones_row")

        def make_consts():
            # emitted after the first gather burst so these gpsimd ops
            # don't sit ahead of the gathers in the queue
            make_identity(nc, ident_sb[:])
            nc.gpsimd.memset(ones_col[:], 1.0)
            nc.gpsimd.memset(ones_row[:], 1.0)

        def qproj(b):
            # q_rep[p, o] = sum_h query[h] * WqT[h, o]   (same for all p)
            qps = qps_pool.tile([128, H], F32, tag="qps")
            for c in range(HC):
                lhsT = qcol_sb[:, b * HC + c : b * HC + c + 1].to_broadcast([128, 128])
                for hh in range(2):
                    nc.tensor.matmul(
                        qps[:, hh * 512 : (hh + 1) * 512],
                        lhsT=lhsT,
                        rhs=wqt_sb[:, c * H + hh * 512 : c * H + (hh + 1) * 512],
                        start=(c == 0),
                        stop=(c == HC - 1),
                    )
            qrep = sm_pool.tile([128, H], F32, tag="qrep")
            nc.scalar.copy(qrep[:], qps[:])
            return qrep

        def fused_tile(b, t, qrep, scores, p_col, p_bf, cps):
            """One s-tile: gather pk+v, score it, exp (mask folded into
            the per-partition exp bias), bf16-cast, and accumulate the
            context matmul in PSUM — fully online softmax (skipping the
            max-subtraction is safe: |scores| <= sum|we| keeps exp in
            fp32 range, matching the reference softmax exactly)."""
            pk_t = pk_pool.tile([128, H], F32, tag="pk")
            if sparse:
                nc.gpsimd.indirect_dma_start(
                    out=pk_t[:],
                    out_offset=None,
                    in_=pk[:],
                    in_offset=bass.IndirectOffsetOnAxis(
                        ap=idx_sb[:, b * T + t : b * T + t + 1], axis=0
                    ),
                )
            else:
                nc.sync.dma_start(pk_t[:], pk[ds(b * S + t * 128, 128), :])
            v_t = v_pool.tile([128, D], F16, tag="v")
            if sparse:
                nc.gpsimd.indirect_dma_start(
                    out=v_t[:],
                    out_offset=None,
                    in_=val[:],
                    in_offset=bass.IndirectOffsetOnAxis(
                        ap=idx_sb[:, b * T + t : b * T + t + 1], axis=0
                    ),
                )
            else:
                nc.gpsimd.dma_start(v_t[:], val[ds(b * S + t * 128, 128), :])
            s_t = sum_pool.tile([128, H], F32, tag="sum")
            nc.vector.tensor_add(s_t[:], pk_t[:], qrep[:])
            e_t = e_pool.tile([128, H], F32, tag="e")
            nc.scalar.activation(e_t[:], s_t[:], AF.Tanh)
            pr_t = pr_pool.tile([128, H], F32, tag="pr")
            nc.vector.scalar_tensor_tensor(
                out=pr_t[:],
                in0=e_t[:],
                scalar=0.0,
                in1=werep_sb[:],
                op0=ALU.bypass,
                op1=ALU.mult,
                accum_out=scores[:, t : t + 1],
            )
            nc.scalar.activation(
                p_col[:, t : t + 1],
                scores[:, t : t + 1],
                AF.Exp,
                bias=nmask_sb[:, b * T + t : b * T + t + 1],
            )
            nc.vector.tensor_copy(p_bf[:, t : t + 1], p_col[:, t : t + 1])
            for cc in range(D // 512):
                nc.tensor.matmul(
                    cps[:, cc * 512 : (cc + 1) * 512],
                    lhsT=p_bf[:, t : t + 1],
                    rhs=v_t[:, cc * 512 : (cc + 1) * 512],
                    start=(t == 0),
                    stop=(t == T - 1),
                )

        def finish_batch(b, p_col, cps):
            rowsum = sm_pool.tile([128, 1], F32, tag="rowsum")
            nc.vector.reduce_sum(rowsum[:], p_col[:], axis=mybir.AxisListType.X)
            zps = sps_pool.tile([1, 1], F32, tag="sps")
            nc.tensor.matmul(
                zps[:], lhsT=rowsum[:], rhs=ones_col[:], start=True, stop=True
            )
            rz = sm_pool.tile([1, 1], F32, tag="rz")
            nc.vector.reciprocal(rz[:], zps[:])
            rzcps = sps_pool.tile([128, 1], F32, tag="sps")
            nc.tensor.matmul(
                rzcps[:], lhsT=ones_row[:], rhs=rz[:], start=True, stop=True
            )
            rzc = sm_pool.tile([128, 1], F32, tag="rzc")
            nc.vector.tensor_copy(rzc[:], rzcps[:])

            ctx_sb = out_pool.tile([1, D], F32, tag="ctx_sb")
            nc.scalar.mul(ctx_sb[:], cps[:], rz[0:1, 0:1])
            nc.sync.dma_start(ctx_out[b : b + 1, :], ctx_sb[:])
            aps = sps_pool.tile([T, 128], F32, tag="sps")
            nc.tensor.transpose(out=aps[:], in_=p_col[:], identity=ident_sb[:])
            at_sb = out_pool.tile([T, 128], F32, tag="at_sb")
            nc.scalar.mul(at_sb[:], aps[:], rzc[0:T, 0:1])
            nc.sync.dma_start(alphat[b, :, :], at_sb[:])

        qreps = [qproj(b) for b in range(BL)]
        for b in range(BL):
            scores = sm_pool.tile([128, T], F32, tag="scores")
            p_col = sm_pool.tile([128, T], F32, tag="pcol")
            p_bf = sm_pool.tile([128, T], F16, tag="pbf")
            cps = cps_pool.tile([1, D], F32, tag="cps")
            for t in range(T):
                fused_tile(b, t, qreps[b], scores, p_col, p_bf, cps)
            if b == 0:
                make_consts()
            finish_batch(b, p_col, cps)

    nc.compile()
    return nc


_program_cache = {}


def _get_program(T: int, sparse: bool):
    key = (T, sparse)
    if key not in _program_cache:
        _program_cache[key] = build_program(T, sparse)
    return _program_cache[key]


def kernel(query, proj_key, value, Wq, we, mask, sparse=True, trace=False):
    query = np.asarray(query, dtype=np.float32)
    proj_key = np.asarray(proj_key, dtype=np.float32)
    value = np.asarray(value, dtype=np.float32)
    Wq = np.asarray(Wq, dtype=np.float32)
    we = np.asarray(we, dtype=np.float32)
    mask = np.asarray(mask)
    mask_i = np.asarray(mask, dtype=np.int64)

    # shared host-side layout prep (replicated params)
    wqt_np = np.ascontiguousarray(Wq.T)
    werep_np = np.ascontiguousarray(np.broadcast_to(we, (128, H)))

    if sparse:
        idx_lists = [np.nonzero(mask_i[g] != 0)[0].astype(np.int32) for g in range(B)]
        kmax = max(1, max(len(ix) for ix in idx_lists))
        T = -(-kmax // 128) * 128 // 128  # ceil to tiles
    else:
        T = S // 128

    nc = _get_program(T, sparse)

    in_maps = []
    for c in range(N_CORES):
        g0 = c * BL
        pk_np = np.ascontiguousarray(proj_key[g0 : g0 + BL].reshape(BL * S, H))
        v_np = np.ascontiguousarray(value[g0 : g0 + BL].reshape(BL * S, D))
        qcol_np = np.ascontiguousarray(
            query[g0 : g0 + BL, 0, :].reshape(BL, HC, 128).transpose(0, 2, 1)
        )
        nm = np.full((BL, T * 128), NEG, dtype=np.float32)
        ix = np.zeros((BL, T * 128), dtype=np.int32)
        for i in range(BL):
            g = g0 + i
            if sparse:
                k = len(idx_lists[g])
                nm[i, :k] = 0.0
                ix[i, :k] = g0 * 0 + i * S + idx_lists[g]
                ix[i, k:] = i * S  # harmless valid row; weight is exactly 0
            else:
                nm[i] = np.where(mask_i[g] != 0, 0.0, NEG).astype(np.float32)
        nm_col = np.ascontiguousarray(
            nm.reshape(BL, T, 128).transpose(0, 2, 1)
        )
        m = {
            "pk": pk_np,
            "val": v_np,
            "wqt": wqt_np,
            "qcol": qcol_np,
            "werep": werep_np,
            "nmask": nm_col,
        }
        if sparse:
            m["idx"] = np.ascontiguousarray(ix.reshape(BL, T, 128).transpose(0, 2, 1))
        in_maps.append(m)

    res = run_bass_kernel_spmd(
        nc, in_maps, core_ids=list(range(N_CORES)), trace=trace
    )

    context = np.zeros((B, 1, D), dtype=np.float32)
    alphas = np.zeros((B, 1, S), dtype=np.float32)
    for c in range(N_CORES):
        r = res.results[c]
        for i in range(BL):
            g = c * BL + i
            context[g, 0, :] = r["ctx"][i]
            vals = r["alphat"][i].reshape(T * 128)
            if sparse:
                k = len(idx_lists[g])
                alphas[g, 0, idx_lists[g]] = vals[:k]
            else:
                alphas[g, 0, :] = vals
    kernel.last_exec_time_ns = res.exec_time_ns
    kernel.last_results = res
    return context, alphas
